# revision 11
# baseline (speedup 1.0000x reference)
"""Trainium2 Bass kernel for a pre-LN transformer block (B=8,T=1024,C=1024,H=16,FF=4096).

Sharding: pure data-parallel over batch — B=8 equals the 8 NeuronCores, each core
runs the full block on one (T, C) slice; weights are replicated. No collectives.

Per-core layout strategy:
  - LayerNorm stats via two scalar-engine accumulating passes (sum, sum-sq);
    normalize on the scalar engine as Identity(x*rstd - mu*rstd) with
    per-partition scale/bias APs. Vector only does the tiny [P,1] arithmetic.
  - QKV in fp8 DoubleRow pairs (2x PE throughput): k/q-proj feature-major bf16
    (S stays bf16), v token-major fp8 with a ones column for softmax denoms.
    1/sqrt(D) folded into the exp scale. qk projections are split by token
    chunk: tc2=0 emitted early (overlapping LN of token tiles 4-7), tc2=1
    interleaved one-per-step with the attention head pipeline to keep the PE
    dense (p-state!) while exp paces the softmax.
  - S^T[j,i] bf16 K=64 matmuls into [P,2,512] paired PSUM tiles so exp runs
    once per j-tile pair; probabilities stored fp8 in per-(ic,h) [P,n_jt,512]
    tiles; PV as fp8 DoubleRow j-tile pairs with the beyond-causal gap of odd
    tiles zeroed on gpsimd. Softmax normalize: denom row copied on gpsimd,
    PE-broadcast, reciprocal_approx_fast + multiply on vector.
  - attn-proj in fp8 DoubleRow; fc / mlp-proj bf16 (fp8 would blow the 2e-2
    error budget); fc tc2=0 overlaps proj/LN2 of token tiles 4-7; residuals
    fp32; mlp tail drains per-token-tile to shorten the final critical path.
"""

import functools

import ml_dtypes
import numpy as np

import concourse.bass as bass
import concourse.mybir as mybir
import concourse.tile as tile
from concourse import bacc
from concourse.bass_utils import run_bass_kernel_spmd

bf16 = ml_dtypes.bfloat16
f8e4 = ml_dtypes.float8_e4m3
FP32 = mybir.dt.float32
BF16 = mybir.dt.bfloat16
F8 = mybir.dt.float8e4
AX = mybir.AxisListType
OP = mybir.AluOpType
AF = mybir.ActivationFunctionType
DR = mybir.MatmulPerfMode.DoubleRow

B, T, C, H = 8, 1024, 1024, 16
D = C // H          # 64
FF = 4 * C          # 4096
P = 128
NT = T // P         # 8 token tiles
NCT = C // P        # 8 channel tiles
NFT = FF // P       # 32 ff tiles
SCALE = 0.125       # 1/sqrt(D), folded into exp


def emit_block(nc, tc):
    x_d = nc.dram_tensor("x", [T, C], FP32, kind="ExternalInput").ap()
    wqk_d = nc.dram_tensor("wqk", [16, P, NCT, P], F8, kind="ExternalInput").ap()
    wv_d = nc.dram_tensor("wv", [P, NCT, C], F8, kind="ExternalInput").ap()
    wproj_d = nc.dram_tensor("wproj", [P, NCT, C], F8, kind="ExternalInput").ap()
    wfc_d = nc.dram_tensor("wfc", [NFT, P, NCT, P], BF16, kind="ExternalInput").ap()
    wmp_d = nc.dram_tensor("wmp", [2, P, NFT, 512], BF16, kind="ExternalInput").ap()
    ident_d = nc.dram_tensor("ident", [P, P], BF16, kind="ExternalInput").ap()
    tri_d = nc.dram_tensor("tri01", [P, P], F8, kind="ExternalInput").ap()
    out_d = nc.dram_tensor("out", [T, C], FP32, kind="ExternalOutput").ap()

    from contextlib import ExitStack
    with ExitStack() as top:
        cpool = top.enter_context(tc.tile_pool(name="const", bufs=1))
        ppool = top.enter_context(tc.tile_pool(name="persist", bufs=1))
        spool = top.enter_context(tc.tile_pool(name="stream", bufs=2))
        sm = top.enter_context(tc.tile_pool(name="small", bufs=4))
        aoT_pool = top.enter_context(tc.tile_pool(name="aoT", bufs=1))
        wp_pool = top.enter_context(tc.tile_pool(name="wproj", bufs=1))
        cs = top.enter_context(ExitStack())
        ps_mm = cs.enter_context(tc.tile_pool(name="ps_mm", bufs=2, space="PSUM"))

        ident = cpool.tile([P, P], BF16, tag="ident")
        tri01 = cpool.tile([P, P], F8, tag="tri01")
        zero1 = cpool.tile([P, 1], FP32, tag="zero1")
        eps1 = cpool.tile([P, 1], FP32, tag="eps1")
        ones_row = cpool.tile([1, P], BF16, tag="ones_row")

        x2_sb = ppool.tile([P, NT, C], FP32, tag="x2")
        aoT = aoT_pool.tile([P, NCT, T], F8, tag="aoT")
        wp = wp_pool.tile([P, NCT, C], F8, tag="wproj")

        def emit_ln(x_tile, h_out):
            width = x_tile.shape[-1]
            s = sm.tile([P, 1], FP32, tag="ln_s")
            ssq = sm.tile([P, 1], FP32, tag="ln_ssq")
            mu = sm.tile([P, 1], FP32, tag="ln_mu")
            var = sm.tile([P, 1], FP32, tag="ln_var")
            std = sm.tile([P, 1], FP32, tag="ln_std")
            rstd = sm.tile([P, 1], FP32, tag="ln_rstd")
            sq = spool.tile([P, C], FP32, tag="ln_sq")
            nc.vector.reduce_sum(s, x_tile, axis=AX.X)
            nc.scalar.activation(sq[:, :width], x_tile, AF.Square, accum_out=ssq)
            nc.vector.tensor_scalar_mul(mu, s, 1.0 / width)
            nc.vector.tensor_scalar_mul(var, ssq, 1.0 / width)
            nc.vector.tensor_tensor(s, mu, mu, op=OP.mult)
            nc.vector.tensor_tensor(var, var, s, op=OP.subtract)
            nc.scalar.activation(std, var, AF.Sqrt, bias=1e-5)
            nc.vector.reciprocal(rstd, std)
            nc.vector.tensor_scalar(h_out, x_tile, scalar1=mu, scalar2=rstd,
                                    op0=OP.subtract, op1=OP.mult)

        with ExitStack() as attn_scope:
            ps_pv = attn_scope.enter_context(
                tc.tile_pool(name="ps_pv", bufs=2, space="PSUM"))
            ps_s = attn_scope.enter_context(
                tc.tile_pool(name="ps_s", bufs=2, space="PSUM"))
            qk_pool = attn_scope.enter_context(tc.tile_pool(name="qk", bufs=1))
            v_pool = attn_scope.enter_context(tc.tile_pool(name="v", bufs=1))
            hT_pool = attn_scope.enter_context(tc.tile_pool(name="hTp", bufs=1))
            wq_pool = attn_scope.enter_context(tc.tile_pool(name="wqkv", bufs=1))
            wv_pool = attn_scope.enter_context(tc.tile_pool(name="wvp", bufs=1))
            pt0_pool = attn_scope.enter_context(tc.tile_pool(name="pt0", bufs=3))
            pt1_pool = attn_scope.enter_context(tc.tile_pool(name="pt1", bufs=3))
            rb_pool = attn_scope.enter_context(tc.tile_pool(name="rbp", bufs=2))

            qpT = qk_pool.tile([P, NCT, T], BF16, tag="qpT")  # q-proj^T (key role)
            kpT = qk_pool.tile([P, NCT, T], BF16, tag="kpT")  # k-proj^T (query role)
            v_aug = v_pool.tile([P, NT, H, 2 * D], F8, tag="vaug")
            hT = hT_pool.tile([P, NCT, T], F8, tag="hT")
            wqk_sb = wq_pool.tile([P, 16, NCT, P], F8, tag="wqk")

            # constants first (first transpose needs ident), then x tiles,
            # wv early (v-proj of tile 0), wp deferred to phase D.
            nc.sync.dma_start(ident[:], ident_d)
            nc.sync.dma_start(tri01[:], tri_d)
            xts = []
            for tt in range(NT):
                xt = spool.tile([P, C], FP32, tag="xin", name=f"xin{tt}")
                xts.append(xt)
            for tt in range(3):
                nc.sync.dma_start(xts[tt][:], x_d[tt * P:(tt + 1) * P, :])
            wv_sb = wv_pool.tile([P, NCT, C], F8, tag="wv")
            nc.sync.dma_start(wv_sb[:], wv_d)
            for tt in range(3, NT):
                nc.sync.dma_start(xts[tt][:], x_d[tt * P:(tt + 1) * P, :])
            nc.gpsimd.memset(zero1[:], 0.0)
            nc.gpsimd.memset(eps1[:], 1e-5)
            nc.gpsimd.memset(ones_row[:], 1.0)
            nc.const_aps.aps[(FP32, 0.0)] = zero1[:]
            nc.const_aps.aps[(FP32, 1e-5)] = eps1[:]
            nc.gpsimd.memset(v_aug[:, :, :, 0:D], 1.0)

            def emit_lnchunk(tt):
                """LN1 + transpose + v-projection for one token tile."""
                ht = spool.tile([P, C], BF16, tag="h")
                emit_ln(xts[tt][:], ht[:])
                for ct in range(NCT):
                    ptr = ps_s.tile([P, P], BF16, tag="smm")
                    nc.tensor.transpose(ptr[:], ht[:, ct * P:(ct + 1) * P], ident[:])
                    nc.vector.tensor_copy(out=hT[:, ct, tt * P:(tt + 1) * P],
                                          in_=ptr[:])
                for fc2 in range(2):
                    pm = ps_mm.tile([P, 512], FP32, tag="mm")
                    for cp in range(NCT // 2):
                        nc.tensor.matmul(
                            pm[:], hT[:, 2 * cp:2 * cp + 2, tt * P:(tt + 1) * P],
                            wv_sb[:, 2 * cp:2 * cp + 2, fc2 * 512:(fc2 + 1) * 512],
                            start=(cp == 0), stop=(cp == NCT // 2 - 1),
                            perf_mode=DR)
                    nc.vector.tensor_copy(
                        out=v_aug[:, tt, fc2 * 8:(fc2 + 1) * 8, D:2 * D],
                        in_=pm[:].rearrange("p (h d) -> p h d", d=D))

            def emit_qk_ft(ft, tc2):
                if tc2 == 0:
                    nc.sync.dma_start(wqk_sb[:, ft], wqk_d[ft])
                dst = kpT if ft < 8 else qpT
                pm = ps_mm.tile([P, 512], FP32, tag="mm", name=f"qk{ft}_{tc2}")
                for cp in range(NCT // 2):
                    nc.tensor.matmul(
                        pm[:], wqk_sb[:, ft, 2 * cp:2 * cp + 2, :],
                        hT[:, 2 * cp:2 * cp + 2, tc2 * 512:(tc2 + 1) * 512],
                        start=(cp == 0), stop=(cp == NCT // 2 - 1),
                        perf_mode=DR)
                nc.vector.tensor_copy(
                    out=dst[:, ft % 8, tc2 * 512:(tc2 + 1) * 512], in_=pm[:])

            # ---- phase A1: token tiles 0-3 ----
            for tt in range(4):
                emit_lnchunk(tt)

            # ---- S1: qk projections tc2=0, overlapping LN of tiles 4-7 ----
            ftlist = [v for hp in range(8) for v in (hp, 8 + hp)]
            for i, ft in enumerate(ftlist):
                emit_qk_ft(ft, 0)
                if i % 4 == 1:
                    emit_lnchunk(4 + i // 4)

            def emit_S_ic(ic, h, ptile):
                po = (h % 2) * D
                cth = h // 2
                n_jt = 4 * ic + 4
                for pj in range(n_jt // 2):
                    j0, j1 = 2 * pj, 2 * pj + 1
                    vs0 = max(0, j0 * P - ic * 512)
                    vs1 = max(0, j1 * P - ic * 512)
                    pm = ps_s.tile([P, 2, 512], FP32, tag="smm",
                                   name=f"s{ic}_{h}_{pj}")
                    for jj in range(2):
                        # both lanes span [vs0:512]: the odd lane's extra
                        # beyond-causal columns are finite junk, zeroed in pt
                        jt = 2 * pj + jj
                        nc.tensor.matmul(
                            pm[:, jj, vs0:512],
                            qpT[po:po + D, cth, jt * P:(jt + 1) * P],
                            kpT[po:po + D, cth, ic * 512 + vs0:(ic + 1) * 512],
                            start=True, stop=True)
                    nc.scalar.activation(ptile[:, j0:j1 + 1, vs0:512],
                                         pm[:, :, vs0:512], AF.Exp, scale=SCALE)
                    if vs1 > vs0:
                        # zero the odd tile's beyond-causal gap (exp of psum
                        # garbage) so the PV pair can read the union width
                        nc.gpsimd.memset(ptile[:, j1, vs0:vs1], 0.0)
                    for jt in (j0, j1):
                        if jt >= ic * 4:
                            dd = jt * P - ic * 512
                            nc.vector.tensor_tensor(
                                ptile[:, jt, dd:dd + P], ptile[:, jt, dd:dd + P],
                                tri01[:], op=OP.mult)

            def emit_PV(h, apair):
                pv_pair = []
                for ic in range(2):
                    pvT = ps_pv.tile([2 * D, 512], FP32, tag="pv",
                                     name=f"pv{ic}_{h}")
                    n_pair = 2 * ic + 2
                    for pj in range(n_pair):
                        vs = max(0, 2 * pj * P - ic * 512)
                        nc.tensor.matmul(
                            pvT[:, vs:512],
                            v_aug[:, 2 * pj:2 * pj + 2, h, :],
                            apair[ic][:, 2 * pj:2 * pj + 2, vs:512],
                            start=(pj == 0), stop=(pj == n_pair - 1),
                            perf_mode=DR)
                    pv_pair.append(pvT)
                return pv_pair

            def emit_PV_norm(ic, h, pvT):
                po = (h % 2) * D
                cth = h // 2
                rcp = rb_pool.tile([D, 512], FP32, tag="rcp")
                nc.vector.reciprocal_approx_fast(rcp[:], pvT[0:D, :])
                nc.vector.tensor_tensor(
                    aoT[po:po + D, cth, ic * 512:(ic + 1) * 512],
                    pvT[D:2 * D, :], rcp[:], op=OP.mult)

            # ---- S3: qk tc2=1 one-per-step + attention head pipeline ----
            pts = {}
            pvs = {}
            for s in range(H + 4):
                if s < 16:
                    emit_qk_ft(ftlist[s], 1)
                h0 = s - 1
                if 0 <= h0 < H:
                    pts[h0] = (
                        pt0_pool.tile([P, 4, 512], F8, tag="pt0", name=f"pt0_{h0}"),
                        pt1_pool.tile([P, 8, 512], F8, tag="pt1", name=f"pt1_{h0}"),
                    )
                    emit_S_ic(0, h0, pts[h0][0])
                h1 = s - 2
                if 0 <= h1 < H:
                    emit_S_ic(1, h1, pts[h1][1])
                h_pv = s - 3
                if 0 <= h_pv < H:
                    pvs[h_pv] = emit_PV(h_pv, pts.pop(h_pv))
                h_n = s - 4
                if 0 <= h_n < H:
                    b0, b1 = pvs.pop(h_n)
                    emit_PV_norm(0, h_n, b0)
                    emit_PV_norm(1, h_n, b1)

        # ---- phase D: attn-proj + LN2 + fc + mlp ----
        if True:
            ps_tr2 = cs.enter_context(
                tc.tile_pool(name="ps_tr2", bufs=2, space="PSUM"))
            ps_fc = cs.enter_context(
                tc.tile_pool(name="ps_fc", bufs=4, space="PSUM"))
            h2_pool = top.enter_context(tc.tile_pool(name="h2Tp", bufs=1))
            mT_pool = top.enter_context(tc.tile_pool(name="mT", bufs=1))
            wf_pool = cs.enter_context(tc.tile_pool(name="wfc", bufs=4))
            h2T = h2_pool.tile([P, NCT, T], BF16, tag="h2T")
            mT = mT_pool.tile([P, NFT, T], BF16, tag="mT")

            nc.sync.dma_start(wp[:], wproj_d)

            def emit_proj(tt):
                for cc2 in range(2):
                    pm = ps_fc.tile([P, 512], FP32, tag="fcp", name=f"prj{tt}_{cc2}")
                    for cp in range(NCT // 2):
                        nc.tensor.matmul(
                            pm[:], aoT[:, 2 * cp:2 * cp + 2, tt * P:(tt + 1) * P],
                            wp[:, 2 * cp:2 * cp + 2, cc2 * 512:(cc2 + 1) * 512],
                            start=(cp == 0), stop=(cp == NCT // 2 - 1),
                            perf_mode=DR)
                    xr = spool.tile([P, 512], FP32, tag="xres")
                    nc.sync.dma_start(
                        xr[:], x_d[tt * P:(tt + 1) * P, cc2 * 512:(cc2 + 1) * 512])
                    nc.vector.tensor_tensor(
                        x2_sb[:, tt, cc2 * 512:(cc2 + 1) * 512], pm[:], xr[:],
                        op=OP.add)

            def emit_ln2(tt):
                h2 = spool.tile([P, C], BF16, tag="h")
                emit_ln(x2_sb[:, tt, :], h2[:])
                for ct in range(NCT):
                    ptr = ps_tr2.tile([P, P], BF16, tag="tr2")
                    nc.tensor.transpose(ptr[:], h2[:, ct * P:(ct + 1) * P], ident[:])
                    nc.vector.tensor_copy(out=h2T[:, ct, tt * P:(tt + 1) * P],
                                          in_=ptr[:])

            for tt in range(4):
                emit_proj(tt)
                emit_ln2(tt)

            def emit_fc(ft, tc2):
                wf = wf_pool.tile([P, NCT, P], BF16, tag="wfc", name=f"wfc{tc2}_{ft}")
                nc.sync.dma_start(wf[:], wfc_d[ft])
                pm = ps_fc.tile([P, 512], FP32, tag="fcp", name=f"fc{tc2}_{ft}")
                for ct in range(NCT):
                    nc.tensor.matmul(pm[:], wf[:, ct, :],
                                     h2T[:, ct, tc2 * 512:(tc2 + 1) * 512],
                                     start=(ct == 0), stop=(ct == NCT - 1))
                nc.scalar.activation(mT[:, ft, tc2 * 512:(tc2 + 1) * 512],
                                     pm[:], AF.Gelu)

            # fc tc2=0 overlaps proj/LN2 of token tiles 4-7
            for ft in range(NFT):
                emit_fc(ft, 0)
                if ft % 8 == 1 and ft // 8 < 4:
                    tt = 4 + ft // 8
                    emit_proj(tt)
                    emit_ln2(tt)
            for ft in range(NFT):
                emit_fc(ft, 1)

            cs.close()  # release mm/fc psum+sbuf before the 8-bank proj pool
            with ExitStack() as pr_scope:
                wm_pool = pr_scope.enter_context(tc.tile_pool(name="wmp", bufs=3))
                ps_pr = pr_scope.enter_context(
                    tc.tile_pool(name="ps_proj", bufs=8, space="PSUM"))
                for cc2 in range(2):
                    pms = [ps_pr.tile([P, 512], FP32, tag="mproj", name=f"mp{cc2}_{i}")
                           for i in range(NT)]
                    for fg in range(NFT // 4):
                        wm = wm_pool.tile([P, 4, 512], BF16, tag="wmp")
                        nc.sync.dma_start(wm[:], wmp_d[cc2][:, fg * 4:(fg + 1) * 4, :])
                        last = fg == NFT // 4 - 1
                        # last group runs token-tile-major so each tile's
                        # accumulation closes early and its drain overlaps
                        order = ([(tt, fi) for tt in range(NT) for fi in range(4)]
                                 if last else
                                 [(tt, fi) for fi in range(4) for tt in range(NT)])
                        for tt, fi in order:
                            ft = fg * 4 + fi
                            nc.tensor.matmul(pms[tt][:],
                                             mT[:, ft, tt * P:(tt + 1) * P],
                                             wm[:, fi, :],
                                             start=(ft == 0), stop=(ft == NFT - 1))
                            if last and fi == 3:
                                ot = spool.tile([P, 512], FP32, tag="osb")
                                nc.vector.tensor_tensor(
                                    ot[:], pms[tt][:],
                                    x2_sb[:, tt, cc2 * 512:(cc2 + 1) * 512],
                                    op=OP.add)
                                nc.sync.dma_start(
                                    out_d[tt * P:(tt + 1) * P,
                                          cc2 * 512:(cc2 + 1) * 512], ot[:])


@functools.lru_cache(maxsize=1)
def _compiled():
    nc = bacc.Bacc("TRN2", target_bir_lowering=False, debug=False)
    with tile.TileContext(nc) as tc:
        emit_block(nc, tc)
    nc.compile()
    return nc


def _prepro(inputs):
    f32 = np.float32
    inp = {k: np.asarray(v, f32) for k, v in inputs.items()}
    g1, b1 = inp["ln1_g"], inp["ln1_b"]
    W = inp["attn_w"] * g1[:, None]
    bias_kqv = inp["attn_b"] + b1 @ inp["attn_w"]
    assert not np.any(bias_kqv), "nonzero attn bias not supported by this build"
    assert not np.any(inp["attn_proj_b"]) and not np.any(inp["fc_b"]) \
        and not np.any(inp["mlp_proj_b"]), "nonzero biases not supported"

    wqk = np.ascontiguousarray(
        W[:, :2 * C].astype(f8e4).reshape(NCT, P, 16, P).transpose(2, 1, 0, 3))
    wv = np.ascontiguousarray(
        W[:, 2 * C:].astype(f8e4).reshape(NCT, P, C).transpose(1, 0, 2))
    wproj = np.ascontiguousarray(
        inp["attn_proj_w"].astype(f8e4).reshape(NCT, P, C).transpose(1, 0, 2))
    wfc = np.ascontiguousarray(
        (inp["fc_w"] * inp["ln2_g"][:, None]).astype(bf16)
        .reshape(NCT, P, NFT, P).transpose(2, 1, 0, 3))
    assert not np.any(inp["ln2_b"]), "nonzero ln2 bias not supported"
    wmp = np.ascontiguousarray(
        inp["mlp_proj_w"].astype(bf16).reshape(NFT, P, 2, 512).transpose(2, 1, 0, 3))
    ident = np.eye(P, dtype=bf16)
    tri01 = np.triu(np.ones((P, P), np.float32)).astype(f8e4)  # 1 where col >= row
    return inp["x"], dict(wqk=wqk, wv=wv, wproj=wproj, wfc=wfc, wmp=wmp,
                          ident=ident, tri01=tri01)


def kernel(**inputs) -> np.ndarray:
    x, weights = _prepro(inputs)
    nc = _compiled()
    in_maps = [{"x": np.ascontiguousarray(x[b]), **weights} for b in range(B)]
    res = run_bass_kernel_spmd(nc, in_maps, list(range(B)))
    return np.stack([res.results[b]["out"] for b in range(B)]).astype(np.float32)


# revision 18
# speedup vs baseline: 1.1002x; 1.1002x over previous
"""Trainium2 Bass kernel for a pre-LN transformer block (B=8,T=1024,C=1024,H=16,FF=4096).

Sharding: pure data-parallel over batch — B=8 equals the 8 NeuronCores, each core
runs the full block on one (T, C) slice; weights are replicated. No collectives.

Layout/schedule strategy:
  - Attention is split by i-chunk: ic0 (score columns = tokens 0-511) runs
    pipelined against the qk tc2=0 projections and LN of token tiles 4-7; ic1
    runs pipelined against the qk tc2=1 projections, with the attn-proj of
    token tiles 0-3 as PE filler in the drain steps, so the softmax exp
    (scalar engine) always has PE work beside it.
  - All activation functions live while attention exp runs come from ONE act
    table set (natural_log_exp_and_others): LayerNorm rstd is computed as
    exp(-0.5*ln(var+eps)) instead of sqrt, avoiding 1.28us act-table reloads
    inside the pipeline. Gelu (fc) is confined to the attention-free X2 phase.
  - QKV / v / attn-proj / PV in fp8 DoubleRow pairs (2x PE); S bf16 (K=64
    gains nothing from fp8); 1/sqrt(D) folded into the exp scale; exp runs
    once per [P,2,512] paired S-PSUM tile; probabilities fp8; PV's softmax
    denominator comes from a 64-wide ones block in v_aug (free: matmul cost
    is N-bound), normalized via reciprocal_approx_fast (PSUM base 0 only —
    the custom DVE op breaks on base-64 PSUM reads) + multiply.
  - fc / mlp-proj bf16 (fp8 would blow the 2e-2 error budget); residuals
    fp32; mlp tail drains per-token-tile to shorten the final critical path.
"""

import functools

import ml_dtypes
import numpy as np

import concourse.bass as bass
import concourse.mybir as mybir
import concourse.tile as tile
from concourse import bacc
from concourse.bass_utils import run_bass_kernel_spmd

bf16 = ml_dtypes.bfloat16
f8e4 = ml_dtypes.float8_e4m3
FP32 = mybir.dt.float32
BF16 = mybir.dt.bfloat16
F8 = mybir.dt.float8e4
AX = mybir.AxisListType
OP = mybir.AluOpType
AF = mybir.ActivationFunctionType
DR = mybir.MatmulPerfMode.DoubleRow

B, T, C, H = 8, 1024, 1024, 16
D = C // H          # 64
FF = 4 * C          # 4096
P = 128
NT = T // P         # 8 token tiles
NCT = C // P        # 8 channel tiles
NFT = FF // P       # 32 ff tiles
SCALE = 0.125       # 1/sqrt(D), folded into exp


def emit_block(nc, tc):
    x_d = nc.dram_tensor("x", [T, C], FP32, kind="ExternalInput").ap()
    wqk_d = nc.dram_tensor("wqk", [16, P, NCT, P], F8, kind="ExternalInput").ap()
    wv_d = nc.dram_tensor("wv", [P, NCT, C], F8, kind="ExternalInput").ap()
    wproj_d = nc.dram_tensor("wproj", [P, NCT, C], F8, kind="ExternalInput").ap()
    wfc_d = nc.dram_tensor("wfc", [NFT, P, NCT, P], BF16, kind="ExternalInput").ap()
    wmp_d = nc.dram_tensor("wmp", [2, P, NFT, 512], BF16, kind="ExternalInput").ap()
    ident_d = nc.dram_tensor("ident", [P, P], BF16, kind="ExternalInput").ap()
    tri_d = nc.dram_tensor("tri01", [P, P], F8, kind="ExternalInput").ap()
    out_d = nc.dram_tensor("out", [T, C], FP32, kind="ExternalOutput").ap()

    from contextlib import ExitStack
    with ExitStack() as top:
        cpool = top.enter_context(tc.tile_pool(name="const", bufs=1))
        ppool = top.enter_context(tc.tile_pool(name="persist", bufs=1))
        spool = top.enter_context(tc.tile_pool(name="stream", bufs=2))
        sm = top.enter_context(tc.tile_pool(name="small", bufs=4))
        aoT_pool = top.enter_context(tc.tile_pool(name="aoT", bufs=1))
        wp_pool = top.enter_context(tc.tile_pool(name="wproj", bufs=1))

        ident = cpool.tile([P, P], BF16, tag="ident")
        tri01 = cpool.tile([P, P], F8, tag="tri01")
        zero1 = cpool.tile([P, 1], FP32, tag="zero1")
        eps1 = cpool.tile([P, 1], FP32, tag="eps1")

        x2_sb = ppool.tile([P, NT, C], FP32, tag="x2")
        aoT = aoT_pool.tile([P, NCT, T], F8, tag="aoT")
        wp = wp_pool.tile([P, NCT, C], F8, tag="wproj")

        def emit_ln(x_tile, h_out):
            width = x_tile.shape[-1]
            s = sm.tile([P, 1], FP32, tag="ln_s")
            ssq = sm.tile([P, 1], FP32, tag="ln_ssq")
            mu = sm.tile([P, 1], FP32, tag="ln_mu")
            var = sm.tile([P, 1], FP32, tag="ln_var")
            lnv = sm.tile([P, 1], FP32, tag="ln_lnv")
            rstd = sm.tile([P, 1], FP32, tag="ln_rstd")
            sq = spool.tile([P, C], BF16, tag="ln_sq")
            nc.vector.reduce_sum(s, x_tile, axis=AX.X)
            nc.scalar.activation(sq[:, :width], x_tile, AF.Square, accum_out=ssq)
            nc.vector.tensor_scalar_mul(mu, s, 1.0 / width)
            nc.vector.tensor_scalar_mul(var, ssq, 1.0 / width)
            nc.vector.tensor_tensor(s, mu, mu, op=OP.mult)
            nc.vector.tensor_tensor(var, var, s, op=OP.subtract)
            # rstd = exp(-0.5*ln(var+eps)): keeps the scalar engine inside the
            # natural_log_exp act set (no 1.28us table reload vs Sqrt)
            nc.scalar.activation(lnv, var, AF.Ln, bias=1e-5)
            nc.scalar.activation(rstd, lnv, AF.Exp, scale=-0.5)
            nc.vector.tensor_scalar(h_out, x_tile, scalar1=mu, scalar2=rstd,
                                    op0=OP.subtract, op1=OP.mult)

        with ExitStack() as attn_scope:
            ps_pv = attn_scope.enter_context(
                tc.tile_pool(name="ps_pv", bufs=2, space="PSUM"))
            ps_s = attn_scope.enter_context(
                tc.tile_pool(name="ps_s", bufs=2, space="PSUM"))
            ps_mm = attn_scope.enter_context(
                tc.tile_pool(name="ps_mm", bufs=2, space="PSUM"))
            qk_pool = attn_scope.enter_context(tc.tile_pool(name="qk", bufs=1))
            v_pool = attn_scope.enter_context(tc.tile_pool(name="v", bufs=1))
            pt1_pool = attn_scope.enter_context(tc.tile_pool(name="pt1", bufs=2))
            rb_pool = attn_scope.enter_context(tc.tile_pool(name="rbp", bufs=1))

            qpT = qk_pool.tile([P, NCT, T], BF16, tag="qpT")  # q-proj^T (key role)
            kpT = qk_pool.tile([P, NCT, T], BF16, tag="kpT")  # k-proj^T (query role)
            v_aug = v_pool.tile([P, NT, H, 2 * D], F8, tag="vaug")

            def emit_S_ic(ic, h, ptile):
                po = (h % 2) * D
                cth = h // 2
                n_jt = 4 * ic + 4
                for pj in range(n_jt // 2):
                    j0, j1 = 2 * pj, 2 * pj + 1
                    vs0 = max(0, j0 * P - ic * 512)
                    vs1 = max(0, j1 * P - ic * 512)
                    pm = ps_s.tile([P, 2, 512], FP32, tag="smm",
                                   name=f"s{ic}_{h}_{pj}")
                    for jj in range(2):
                        # both lanes span [vs0:512]: the odd lane's extra
                        # beyond-causal columns are finite junk, zeroed in pt
                        jt = 2 * pj + jj
                        nc.tensor.matmul(
                            pm[:, jj, vs0:512],
                            qpT[po:po + D, cth, jt * P:(jt + 1) * P],
                            kpT[po:po + D, cth, ic * 512 + vs0:(ic + 1) * 512],
                            start=True, stop=True)
                    nc.scalar.activation(ptile[:, j0:j1 + 1, vs0:512],
                                         pm[:, :, vs0:512], AF.Exp, scale=SCALE)
                    if vs1 > vs0:
                        nc.gpsimd.memset(ptile[:, j1, vs0:vs1], 0.0)
                    for jt in (j0, j1):
                        if jt >= ic * 4:
                            dd = jt * P - ic * 512
                            nc.vector.tensor_tensor(
                                ptile[:, jt, dd:dd + P], ptile[:, jt, dd:dd + P],
                                tri01[:], op=OP.mult)

            def emit_PV_ic(ic, h, ptile):
                pvT = ps_pv.tile([2 * D, 512], FP32, tag="pv", name=f"pv{ic}_{h}")
                n_pair = 2 * ic + 2
                for pj in range(n_pair):
                    vs = max(0, 2 * pj * P - ic * 512)
                    nc.tensor.matmul(
                        pvT[:, vs:512],
                        v_aug[:, 2 * pj:2 * pj + 2, h, :],
                        ptile[:, 2 * pj:2 * pj + 2, vs:512],
                        start=(pj == 0), stop=(pj == n_pair - 1),
                        perf_mode=DR)
                return pvT

            def emit_PV_norm(ic, h, pvT):
                po = (h % 2) * D
                cth = h // 2
                rcp = rb_pool.tile([D, 512], FP32, tag="rcp")
                nc.vector.reciprocal_approx_fast(rcp[:], pvT[0:D, :])
                nc.vector.tensor_tensor(
                    aoT[po:po + D, cth, ic * 512:(ic + 1) * 512],
                    pvT[D:2 * D, :], rcp[:], op=OP.mult)

            def emit_proj(tt):
                for cc2 in range(2):
                    pm = ps_mm.tile([P, 512], FP32, tag="mm", name=f"prj{tt}_{cc2}")
                    for cp in range(NCT // 2):
                        nc.tensor.matmul(
                            pm[:], aoT[:, 2 * cp:2 * cp + 2, tt * P:(tt + 1) * P],
                            wp[:, 2 * cp:2 * cp + 2, cc2 * 512:(cc2 + 1) * 512],
                            start=(cp == 0), stop=(cp == NCT // 2 - 1),
                            perf_mode=DR)
                    xr = spool.tile([P, 512], FP32, tag="xres")
                    nc.sync.dma_start(
                        xr[:], x_d[tt * P:(tt + 1) * P, cc2 * 512:(cc2 + 1) * 512])
                    nc.vector.tensor_tensor(
                        x2_sb[:, tt, cc2 * 512:(cc2 + 1) * 512], pm[:], xr[:],
                        op=OP.add)

            with ExitStack() as scope0:
                xin_pool = scope0.enter_context(tc.tile_pool(name="xin", bufs=1))
                hT_pool = scope0.enter_context(tc.tile_pool(name="hTp", bufs=1))
                wq_pool = scope0.enter_context(tc.tile_pool(name="wqkv", bufs=1))
                wv_pool = scope0.enter_context(tc.tile_pool(name="wvp", bufs=1))
                pt0_pool = scope0.enter_context(tc.tile_pool(name="pt0", bufs=3))

                hT = hT_pool.tile([P, NCT, T], F8, tag="hT")
                wqk_sb = wq_pool.tile([P, 16, NCT, P], F8, tag="wqk")

                # constants first (first transpose needs ident), then x tiles,
                # wv early (v-proj of tile 0), wp late (needed only at proj).
                nc.sync.dma_start(ident[:], ident_d)
                nc.sync.dma_start(tri01[:], tri_d)
                xts = []
                for tt in range(NT):
                    xt = xin_pool.tile([P, C], FP32, tag="xin", name=f"xin{tt}")
                    xts.append(xt)
                for tt in range(3):
                    nc.sync.dma_start(xts[tt][:], x_d[tt * P:(tt + 1) * P, :])
                wv_sb = wv_pool.tile([P, NCT, C], F8, tag="wv")
                nc.sync.dma_start(wv_sb[:], wv_d)
                for tt in range(3, NT):
                    nc.sync.dma_start(xts[tt][:], x_d[tt * P:(tt + 1) * P, :])
                nc.gpsimd.memset(zero1[:], 0.0)
                nc.gpsimd.memset(eps1[:], 1e-5)
                nc.const_aps.aps[(FP32, 0.0)] = zero1[:]
                nc.const_aps.aps[(FP32, 1e-5)] = eps1[:]
                nc.gpsimd.memset(v_aug[:, :, :, 0:D], 1.0)

                def emit_lnchunk(tt):
                    """LN1 + transpose + v projection for one token tile."""
                    ht = spool.tile([P, C], BF16, tag="h")
                    emit_ln(xts[tt][:], ht[:])
                    for ct in range(NCT):
                        ptr = ps_s.tile([P, P], BF16, tag="smm")
                        nc.tensor.transpose(ptr[:], ht[:, ct * P:(ct + 1) * P],
                                            ident[:])
                        nc.vector.tensor_copy(out=hT[:, ct, tt * P:(tt + 1) * P],
                                              in_=ptr[:])
                    for fc2 in range(2):
                        pm = ps_mm.tile([P, 512], FP32, tag="mm")
                        for cp in range(NCT // 2):
                            nc.tensor.matmul(
                                pm[:], hT[:, 2 * cp:2 * cp + 2, tt * P:(tt + 1) * P],
                                wv_sb[:, 2 * cp:2 * cp + 2,
                                      fc2 * 512:(fc2 + 1) * 512],
                                start=(cp == 0), stop=(cp == NCT // 2 - 1),
                                perf_mode=DR)
                        nc.vector.tensor_copy(
                            out=v_aug[:, tt, fc2 * 8:(fc2 + 1) * 8, D:2 * D],
                            in_=pm[:].rearrange("p (h d) -> p h d", d=D))

                def emit_qk_ft(ft, tc2):
                    if tc2 == 0:
                        nc.sync.dma_start(wqk_sb[:, ft], wqk_d[ft])
                    dst = kpT if ft < 8 else qpT
                    pm = ps_mm.tile([P, 512], FP32, tag="mm", name=f"qk{ft}_{tc2}")
                    for cp in range(NCT // 2):
                        nc.tensor.matmul(
                            pm[:], wqk_sb[:, ft, 2 * cp:2 * cp + 2, :],
                            hT[:, 2 * cp:2 * cp + 2, tc2 * 512:(tc2 + 1) * 512],
                            start=(cp == 0), stop=(cp == NCT // 2 - 1),
                            perf_mode=DR)
                    nc.vector.tensor_copy(
                        out=dst[:, ft % 8, tc2 * 512:(tc2 + 1) * 512], in_=pm[:])

                # ---- A1: token tiles 0-3 ----
                for tt in range(4):
                    emit_lnchunk(tt)
                nc.sync.dma_start(wp[:], wproj_d)

                # ---- X0: qk tc2=0 + LN 4-7 + full ic0 attention pipeline ----
                ftlist = [v for hp in range(8) for v in (hp, 8 + hp)]
                pt0s = {}
                pv0s = {}

                def ic0_step(hp):
                    """S0 for head pair hp, PV0 for hp-1, norm0 for hp-2."""
                    if 0 <= hp < 8:
                        for h in (2 * hp, 2 * hp + 1):
                            ptile = pt0_pool.tile([P, 4, 512], F8, tag="pt0",
                                                  name=f"pt0_{h}")
                            pt0s[h] = ptile
                            emit_S_ic(0, h, ptile)
                    if 0 <= hp - 1 < 8:
                        for h in (2 * hp - 2, 2 * hp - 1):
                            pv0s[h] = emit_PV_ic(0, h, pt0s.pop(h))
                    if 0 <= hp - 2 < 8:
                        for h in (2 * hp - 4, 2 * hp - 3):
                            emit_PV_norm(0, h, pv0s.pop(h))

                for hp in range(8):
                    emit_qk_ft(ftlist[2 * hp], 0)
                    emit_qk_ft(ftlist[2 * hp + 1], 0)
                    if hp % 2 == 1:
                        emit_lnchunk(4 + hp // 2)
                    ic0_step(hp)

                # ---- X1: qk tc2=1 one-per-step + ic1 attention pipeline +
                #      attn-proj of token tiles 0-3 as drain-step filler ----
                pt1s = {}
                pv1s = {}
                for s in range(H + 4):
                    if s < 16:
                        emit_qk_ft(ftlist[s], 1)
                    if s == 0:
                        ic0_step(8)
                    if s == 1:
                        ic0_step(9)
                    h1 = s - 2
                    if 0 <= h1 < H:
                        ptile = pt1_pool.tile([P, 8, 512], F8, tag="pt1",
                                              name=f"pt1_{h1}")
                        pt1s[h1] = ptile
                        emit_S_ic(1, h1, ptile)
                    h_pv = s - 3
                    if 0 <= h_pv < H:
                        pv1s[h_pv] = emit_PV_ic(1, h_pv, pt1s.pop(h_pv))
                    h_n = s - 4
                    if 0 <= h_n < H:
                        emit_PV_norm(1, h_n, pv1s.pop(h_n))
                    if 16 <= s:
                        emit_proj(s - 16)

        # ---- X2: proj 4-7 + LN2 all + fc (dense, gelu set) ----
        mT_pool = top.enter_context(tc.tile_pool(name="mT", bufs=1))
        mT = mT_pool.tile([P, NFT, T], BF16, tag="mT")
        with ExitStack() as x2_scope:
            h2_pool = x2_scope.enter_context(tc.tile_pool(name="h2Tp", bufs=1))
            wf_pool = x2_scope.enter_context(tc.tile_pool(name="wfc", bufs=4))
            ps_fc = x2_scope.enter_context(
                tc.tile_pool(name="ps_fc", bufs=4, space="PSUM"))
            h2T = h2_pool.tile([P, NCT, T], BF16, tag="h2T")

            def emit_proj2(tt):
                for cc2 in range(2):
                    pm = ps_fc.tile([P, 512], FP32, tag="fcp", name=f"prj{tt}_{cc2}")
                    for cp in range(NCT // 2):
                        nc.tensor.matmul(
                            pm[:], aoT[:, 2 * cp:2 * cp + 2, tt * P:(tt + 1) * P],
                            wp[:, 2 * cp:2 * cp + 2, cc2 * 512:(cc2 + 1) * 512],
                            start=(cp == 0), stop=(cp == NCT // 2 - 1),
                            perf_mode=DR)
                    xr = spool.tile([P, 512], FP32, tag="xres")
                    nc.sync.dma_start(
                        xr[:], x_d[tt * P:(tt + 1) * P, cc2 * 512:(cc2 + 1) * 512])
                    nc.vector.tensor_tensor(
                        x2_sb[:, tt, cc2 * 512:(cc2 + 1) * 512], pm[:], xr[:],
                        op=OP.add)

            def emit_ln2(tt):
                h2 = spool.tile([P, C], BF16, tag="h")
                emit_ln(x2_sb[:, tt, :], h2[:])
                for ct in range(NCT):
                    ptr = ps_fc.tile([P, P], BF16, tag="fcp")
                    nc.tensor.transpose(ptr[:], h2[:, ct * P:(ct + 1) * P], ident[:])
                    nc.vector.tensor_copy(out=h2T[:, ct, tt * P:(tt + 1) * P],
                                          in_=ptr[:])

            def emit_fc(ft, tc2):
                wf = wf_pool.tile([P, NCT, P], BF16, tag="wfc", name=f"wfc{tc2}_{ft}")
                nc.sync.dma_start(wf[:], wfc_d[ft])
                pm = ps_fc.tile([P, 512], FP32, tag="fcp", name=f"fc{tc2}_{ft}")
                for ct in range(NCT):
                    nc.tensor.matmul(pm[:], wf[:, ct, :],
                                     h2T[:, ct, tc2 * 512:(tc2 + 1) * 512],
                                     start=(ct == 0), stop=(ct == NCT - 1))
                nc.scalar.activation(mT[:, ft, tc2 * 512:(tc2 + 1) * 512],
                                     pm[:], AF.Gelu)

            for tt in range(4):
                emit_ln2(tt)
            for tt in range(4, NT):
                emit_proj2(tt)
                emit_ln2(tt)
            for tc2 in range(2):
                for ft in range(NFT):
                    emit_fc(ft, tc2)

        # ---- X3: mlp-proj + residual + store ----
        with ExitStack() as pr_scope:
            wm_pool = pr_scope.enter_context(tc.tile_pool(name="wmp", bufs=3))
            ps_pr = pr_scope.enter_context(
                tc.tile_pool(name="ps_proj", bufs=8, space="PSUM"))
            for cc2 in range(2):
                pms = [ps_pr.tile([P, 512], FP32, tag="mproj", name=f"mp{cc2}_{i}")
                       for i in range(NT)]
                for fg in range(NFT // 4):
                    wm = wm_pool.tile([P, 4, 512], BF16, tag="wmp")
                    nc.sync.dma_start(wm[:], wmp_d[cc2][:, fg * 4:(fg + 1) * 4, :])
                    last = fg == NFT // 4 - 1
                    # last group runs token-tile-major so each tile's
                    # accumulation closes early and its drain overlaps
                    order = ([(tt, fi) for tt in range(NT) for fi in range(4)]
                             if last else
                             [(tt, fi) for fi in range(4) for tt in range(NT)])
                    for tt, fi in order:
                        ft = fg * 4 + fi
                        nc.tensor.matmul(pms[tt][:],
                                         mT[:, ft, tt * P:(tt + 1) * P],
                                         wm[:, fi, :],
                                         start=(ft == 0), stop=(ft == NFT - 1))
                        if last and fi == 3:
                            ot = spool.tile([P, 512], FP32, tag="osb")
                            nc.vector.tensor_tensor(
                                ot[:], pms[tt][:],
                                x2_sb[:, tt, cc2 * 512:(cc2 + 1) * 512],
                                op=OP.add)
                            nc.sync.dma_start(
                                out_d[tt * P:(tt + 1) * P,
                                      cc2 * 512:(cc2 + 1) * 512], ot[:])


@functools.lru_cache(maxsize=1)
def _compiled():
    nc = bacc.Bacc("TRN2", target_bir_lowering=False, debug=False)
    with tile.TileContext(nc) as tc:
        emit_block(nc, tc)
    nc.compile()
    return nc


def _prepro(inputs):
    f32 = np.float32
    inp = {k: np.asarray(v, f32) for k, v in inputs.items()}
    g1, b1 = inp["ln1_g"], inp["ln1_b"]
    W = inp["attn_w"] * g1[:, None]
    bias_kqv = inp["attn_b"] + b1 @ inp["attn_w"]
    assert not np.any(bias_kqv), "nonzero attn bias not supported by this build"
    assert not np.any(inp["attn_proj_b"]) and not np.any(inp["fc_b"]) \
        and not np.any(inp["mlp_proj_b"]), "nonzero biases not supported"

    wqk = np.ascontiguousarray(
        W[:, :2 * C].astype(f8e4).reshape(NCT, P, 16, P).transpose(2, 1, 0, 3))
    wv = np.ascontiguousarray(
        W[:, 2 * C:].astype(f8e4).reshape(NCT, P, C).transpose(1, 0, 2))
    wproj = np.ascontiguousarray(
        inp["attn_proj_w"].astype(f8e4).reshape(NCT, P, C).transpose(1, 0, 2))
    wfc = np.ascontiguousarray(
        (inp["fc_w"] * inp["ln2_g"][:, None]).astype(bf16)
        .reshape(NCT, P, NFT, P).transpose(2, 1, 0, 3))
    assert not np.any(inp["ln2_b"]), "nonzero ln2 bias not supported"
    wmp = np.ascontiguousarray(
        inp["mlp_proj_w"].astype(bf16).reshape(NFT, P, 2, 512).transpose(2, 1, 0, 3))
    ident = np.eye(P, dtype=bf16)
    tri01 = np.triu(np.ones((P, P), np.float32)).astype(f8e4)  # 1 where col >= row
    return inp["x"], dict(wqk=wqk, wv=wv, wproj=wproj, wfc=wfc, wmp=wmp,
                          ident=ident, tri01=tri01)


def kernel(**inputs) -> np.ndarray:
    x, weights = _prepro(inputs)
    nc = _compiled()
    in_maps = [{"x": np.ascontiguousarray(x[b]), **weights} for b in range(B)]
    res = run_bass_kernel_spmd(nc, in_maps, list(range(B)))
    return np.stack([res.results[b]["out"] for b in range(B)]).astype(np.float32)
